# revision 1
# baseline (speedup 1.0000x reference)
"""Trainium2 Bass kernel: 2-layer MLP forward  y = relu(x@W1 + b1) @ W2 + b2.

Shapes: x [262144, 64], W1 [64, 128], b1 [128], W2 [128, 32], b2 [32].
Strategy (pure data parallel over 8 NeuronCores, 32768 rows each):

  * Host pre-transposes each x shard to feature-major xt [64, 32768] so the
    contraction dim lands on SBUF partitions (PE matmul contracts over the
    partition dim; a row-major x would otherwise need an on-chip transpose).
  * Device processes 2048-row super-chunks:
      - one 1 MiB DMA loads xt chunk as [128, 1024]: partitions 0-63 hold
        features of rows [C, C+1024), partitions 64-127 rows [C+1024, C+2048)
      - 4x matmul (K=64, alternating PE row-groups 0-1/2-3): lhsT = W1
        (stacked twice on 128 partitions), rhs = xt slices -> h_T in PSUM
      - ScalarE activation: relu(h + b1), PSUM -> SBUF
      - 4x matmul (K=128, col-tiled): lhsT = W2 at col-groups 0..3, each
        writing a 32-partition slice of one PSUM bank -> y_T stacked [128,512]
      - VectorE: + b2 (per-partition scalar), PSUM -> SBUF
      - DMA out to y_dev [16, 128, 512]
  * Matmuls run as float32r (1 col/cycle vs 4 for plain fp32; data is
    bit-identical fp32, only the instruction dtype differs via AP bitcast).
  * Host un-permutes y_dev back to [32768, 32] per shard and concatenates.
"""

import os
import sys

import numpy as np

if "/opt/trn_rl_repo" not in sys.path:
    sys.path.insert(0, "/opt/trn_rl_repo")

N_CORES = 8
B = 262144
B_C = B // N_CORES  # 32768
N_IN, N_MID, N_OUT = 64, 128, 32
CHUNK = 2048  # rows per super-chunk
QROWS = 512  # rows per matmul (PSUM bank free dim)
N_SC = B_C // CHUNK  # 16

# matmul instruction dtype: "f32r" (fast, ~tf32ish?) or "f32" (4x slower)
MM_DT = os.environ.get("BASS_MLP_MMDT", "f32r")

_CACHE: dict = {}


def _build_nc(mm_dt: str):
    from contextlib import ExitStack

    import concourse.bass as bass  # noqa: F401
    import concourse.tile as tile
    from concourse import bacc, mybir

    f32 = mybir.dt.float32
    bf16 = mybir.dt.bfloat16
    mmdt = {"f32r": mybir.dt.float32r, "f32": f32}[mm_dt]

    nc = bacc.Bacc(
        "TRN2", target_bir_lowering=False, debug=False, num_devices=N_CORES
    )
    # the x/W path is declared in the matmul dtype end-to-end (same 4-byte
    # fp32 payload for f32r; walrus requires matmul operands to be *produced*
    # as float32r, so the DMAs/activations must carry the tag).
    xt_d = nc.dram_tensor("xt", [N_IN, B_C], mmdt, kind="ExternalInput").ap()
    w1_d = nc.dram_tensor("w1", [N_IN, N_MID], mmdt, kind="ExternalInput").ap()
    b1_d = nc.dram_tensor("b1", [N_MID, 1], f32, kind="ExternalInput").ap()
    w2_d = nc.dram_tensor("w2", [N_MID, N_OUT], bf16, kind="ExternalInput").ap()
    b2s_d = nc.dram_tensor("b2s", [N_MID, 1], f32, kind="ExternalInput").ap()
    y_d = nc.dram_tensor(
        "y", [N_SC, N_MID, QROWS], f32, kind="ExternalOutput"
    ).ap()

    with tile.TileContext(nc) as tc, ExitStack() as ctx:
        consts = ctx.enter_context(tc.tile_pool(name="consts", bufs=1))
        x_pool = ctx.enter_context(tc.tile_pool(name="xp", bufs=4))
        h_pool = ctx.enter_context(tc.tile_pool(name="hp", bufs=6))
        y_pool = ctx.enter_context(tc.tile_pool(name="yp", bufs=4))
        hps_pool = ctx.enter_context(tc.tile_pool(name="hps", bufs=5, space="PSUM"))
        yps_pool = ctx.enter_context(tc.tile_pool(name="yps", bufs=2, space="PSUM"))

        # W1 stacked twice on the partition dim so row-groups 0-1 and 2-3 can
        # both serve K=64 matmuls whose rhs lives at base partition 0 / 64.
        w1_t = consts.tile([2 * N_IN, N_MID], mmdt, name="w1_t")
        nc.sync.dma_start(out=w1_t[0:N_IN, :], in_=w1_d)
        nc.sync.dma_start(out=w1_t[N_IN : 2 * N_IN, :], in_=w1_d)
        w2_t = consts.tile([N_MID, N_OUT], bf16, name="w2_t")
        nc.sync.dma_start(out=w2_t[:], in_=w2_d)
        b1_t = consts.tile([N_MID, 1], f32, name="b1_t")
        nc.sync.dma_start(out=b1_t[:], in_=b1_d)
        b2_t = consts.tile([N_MID, 1], f32, name="b2_t")
        nc.sync.dma_start(out=b2_t[:], in_=b2s_d)

        for s in range(N_SC):
            xt_t = x_pool.tile([128, CHUNK // 2], mmdt, name="xt_t", tag="xt")
            half_cols = CHUNK // 2
            for c in range(2):
                nc.sync.dma_start(
                    out=xt_t[64 * c : 64 * (c + 1), :],
                    in_=xt_d[:, s * CHUNK + c * half_cols : s * CHUNK + (c + 1) * half_cols],
                )

            y_ps = yps_pool.tile([128, QROWS], f32, name="y_ps", tag="y_ps")
            for q in range(4):
                c, half = q // 2, q % 2
                h_ps = hps_pool.tile([128, QROWS], f32, name="h_ps", tag="h_ps")
                rhs = xt_t[c * 64 : (c + 1) * 64, half * QROWS : (half + 1) * QROWS]
                lhsT = w1_t[c * 64 : (c + 1) * 64, :]
                nc.tensor.matmul(
                    h_ps[:], lhsT, rhs,
                    start=True, stop=True,
                )
                h_sb = h_pool.tile([128, QROWS], bf16, name="h_sb", tag="h_sb")
                nc.scalar.activation(
                    h_sb[:], h_ps[:],
                    mybir.ActivationFunctionType.Relu, bias=b1_t[:],
                )
                nc.tensor.matmul(
                    y_ps[32 * q : 32 * (q + 1), :],
                    w2_t[:], h_sb[:],
                    start=True, stop=True, tile_position=(0, 32 * q),
                )
            y_sb = y_pool.tile([128, QROWS], f32, name="y_sb", tag="y_sb")
            nc.vector.tensor_scalar_add(y_sb[:], y_ps[:], b2_t[:])
            nc.sync.dma_start(out=y_d[s], in_=y_sb[:])

    nc.compile()
    return nc


def _get_nc(mm_dt: str = MM_DT):
    if mm_dt not in _CACHE:
        _CACHE[mm_dt] = _build_nc(mm_dt)
    return _CACHE[mm_dt]


def _prep_in_maps(x, W1, b1, W2, b2):
    x = np.ascontiguousarray(x, dtype=np.float32)
    # [8, 64, B_C] feature-major shards
    xt = np.ascontiguousarray(x.reshape(N_CORES, B_C, N_IN).transpose(0, 2, 1))
    w1 = np.ascontiguousarray(W1, dtype=np.float32)
    import ml_dtypes
    w2 = np.ascontiguousarray(W2, dtype=np.float32).astype(ml_dtypes.bfloat16)
    b1c = np.ascontiguousarray(b1, dtype=np.float32).reshape(N_MID, 1)
    b2s = np.tile(np.asarray(b2, dtype=np.float32), 4).reshape(N_MID, 1)
    return [
        {"xt": xt[i], "w1": w1, "b1": b1c, "w2": w2, "b2s": b2s}
        for i in range(N_CORES)
    ]


def _unshard(results):
    outs = []
    for i in range(N_CORES):
        yd = results[i]["y"]  # [N_SC, 128, QROWS]
        # yd[s, 32q+o, j] = y[CHUNK*s + QROWS*q + j, o]
        y = (
            yd.reshape(N_SC, 4, N_OUT, QROWS)
            .transpose(0, 1, 3, 2)
            .reshape(B_C, N_OUT)
        )
        outs.append(y)
    return np.ascontiguousarray(np.concatenate(outs, axis=0))


def run(x, W1, b1, W2, b2, trace=False, mm_dt: str = MM_DT):
    from concourse.bass_utils import run_bass_kernel_spmd

    nc = _get_nc(mm_dt)
    in_maps = _prep_in_maps(x, W1, b1, W2, b2)
    res = run_bass_kernel_spmd(nc, in_maps, list(range(N_CORES)), trace=trace)
    return _unshard(res.results), res


def kernel(x, W1, b1, W2, b2):
    y, _ = run(x, W1, b1, W2, b2, trace=False)
    return y



# revision 23
# speedup vs baseline: 868134.1479x; 868134.1479x over previous
"""Trainium2 Bass kernel: 2-layer MLP forward  y = relu(x@W1 + b1) @ W2 + b2.

Shapes: x [262144, 64], W1 [64, 128], b1 [128], W2 [128, 32], b2 [32].
Pure data parallel over 8 NeuronCores, 32768 rows per core.

Per-core dataflow (32 chunks of 1024 rows):
  * Host pre-transposes the x shard to feature-major xt [64, 32768] bf16.
  * xt is DMA'd in 4-chunk quanta (8 big DMAs: the ~790ns fixed per-DMA
    sequencer cost dominates small transfers).
  * mm1 (W1 stationary, xt moving): h_ps [128 mid, 1024 rows] PSUM tile.
  * relu+b1 PSUM->SBUF bf16, round-robined between ScalarE (activation
    with per-partition bias) and VectorE (tensor_scalar (h+b1) max 0)
    to balance engine load (GPSIMD has no PSUM port, it cannot help).
  * mm2 (h block stationary, W2 moving): lhsT = h_sb[:, 128j:128j+128],
    rhs = W2 [128, 32] -> y_ps[128 rows, 32]: 32 PE cycles per 128 rows
    instead of 128 (the moving operand is tiny W2, not h).
  * y: each PSUM bank holds 2 chunks (2048 rows) of outputs. Two paths:
      - DVE path: tensor_tensor add of a pre-tiled b2 pattern,
        PSUM->SBUF bf16, two banks batched per output DMA.
      - direct path (DIRECT_Y banks): bias pre-loaded into the PSUM bank
        by a f32r ones[1,128] x b2t[1,512] matmul (exact f32 bits), mm2
        accumulates on top, bank DMA'd straight PSUM->DRAM f32 from the
        Pool engine's SWDGE queue. Removes those columns from DVE.
  * Chunks are software-pipelined: mm2 for chunk s-1 issues after mm1
    for chunk s, so the PE never waits on the relu engines.
"""

import os
import sys

import numpy as np

if "/opt/trn_rl_repo" not in sys.path:
    sys.path.insert(0, "/opt/trn_rl_repo")

N_CORES = 8
B = 262144
B_C = B // N_CORES  # 32768
N_IN, N_MID, N_OUT = 64, 128, 32
CHUNK = 1024  # rows per chunk (one 2-bank h PSUM tile)
QROWS = 512  # rows per mm1 matmul / y PSUM bank free dim
N_CH = B_C // CHUNK  # 32 chunks
N_YG = B_C // (2 * CHUNK)  # 16 y groups (one PSUM bank per 2 chunks)
# xt DMA schedule: (queue, chunks) in chunk order; SP and Pool run concurrently
X_SCHED = [
    ("sync", 1), ("sync", 1), ("gpsimd", 2), ("sync", 2), ("gpsimd", 2),
    ("sync", 4), ("gpsimd", 4), ("sync", 4), ("gpsimd", 4), ("sync", 4),
    ("gpsimd", 4),
]

# precision mode: "fast" = bf16 x / bf16 y, "precise" = f32r x / f32 y
MODE = os.environ.get("BASS_MLP_MODE", "fast")
# number of relu tiles (out of N_CH) handled by ACT; rest go to DVE
ACT_RELU = int(os.environ.get("BASS_MLP_ACT_RELU", "22"))
# number of y groups (out of N_YG) evacuated by ACT as Copy (bias via PE)
ACT_Y = int(os.environ.get("BASS_MLP_ACT_Y", "0"))

_CACHE: dict = {}


def _spread(n_slots: int, n_pick: int) -> list:
    """Evenly spread n_pick True slots over n_slots (Bresenham)."""
    out, err = [], 0
    for _ in range(n_slots):
        err += n_pick
        if err >= n_slots:
            err -= n_slots
            out.append(True)
        else:
            out.append(False)
    return out


def _build_nc(mode: str, act_relu: int, act_y: int):
    from contextlib import ExitStack

    import concourse.bass as bass  # noqa: F401
    import concourse.tile as tile
    from concourse import bacc, mybir

    f32 = mybir.dt.float32
    f32r = mybir.dt.float32r
    bf16 = mybir.dt.bfloat16
    x_dt = bf16 if mode == "fast" else f32r
    y_dt = bf16 if mode == "fast" else f32
    add = mybir.AluOpType.add
    mx = mybir.AluOpType.max

    acty = [g >= N_YG - act_y for g in range(N_YG)]  # tail groups -> ACT
    relu_eng = _spread(N_CH, act_relu)  # True -> ACT

    nc = bacc.Bacc(
        "TRN2", target_bir_lowering=False, debug=False, num_devices=N_CORES
    )
    xt_d = nc.dram_tensor("xt", [N_IN, B_C], x_dt, kind="ExternalInput").ap()
    w1_d = nc.dram_tensor("w1", [N_IN, N_MID], x_dt, kind="ExternalInput").ap()
    b1_d = nc.dram_tensor("b1", [N_MID, 1], f32, kind="ExternalInput").ap()
    w2_d = nc.dram_tensor("w2", [N_MID, N_OUT], bf16, kind="ExternalInput").ap()
    b2t_d = nc.dram_tensor("b2t", [N_MID, QROWS], bf16, kind="ExternalInput").ap()
    onesr_d = nc.dram_tensor("onesr", [1, N_MID], f32r, kind="ExternalInput").ap()
    b2r_d = nc.dram_tensor("b2r", [1, QROWS], f32r, kind="ExternalInput").ap()
    # pairs of y groups batched per DMA
    y_d = nc.dram_tensor(
        "y", [N_YG // 2, N_MID, 2 * QROWS], y_dt, kind="ExternalOutput"
    ).ap()

    with tile.TileContext(nc) as tc, ExitStack() as ctx:
        consts = ctx.enter_context(tc.tile_pool(name="consts", bufs=1))
        x_pool = ctx.enter_context(tc.tile_pool(name="xp", bufs=4))
        hsb_pool = ctx.enter_context(tc.tile_pool(name="hsb", bufs=5))
        ysb_pool = ctx.enter_context(tc.tile_pool(name="ysb", bufs=3))
        hps_pool = ctx.enter_context(tc.tile_pool(name="hps", bufs=3, space="PSUM"))
        yps_pool = ctx.enter_context(tc.tile_pool(name="yps", bufs=2, space="PSUM"))

        # consts are DMA'd from the (otherwise idle at startup) ACT queue so
        # the SP queue can start streaming xt immediately
        w1_t = consts.tile([N_IN, N_MID], x_dt, name="w1_t")
        nc.sync.dma_start(out=w1_t[:], in_=w1_d)
        w2_t = consts.tile([N_MID, N_OUT], bf16, name="w2_t")
        nc.gpsimd.dma_start(out=w2_t[:], in_=w2_d)
        b1_t = consts.tile([N_MID, 1], f32, name="b1_t")
        nc.sync.dma_start(out=b1_t[:], in_=b1_d)
        b2t_t = consts.tile([N_MID, QROWS], bf16, name="b2t_t")
        nc.gpsimd.dma_start(out=b2t_t[:], in_=b2t_d)
        onesr_t = b2r_t = None
        if act_y:
            onesr_t = consts.tile([1, N_MID], f32r, name="onesr_t")
            nc.gpsimd.dma_start(out=onesr_t[:], in_=onesr_d)
            b2r_t = consts.tile([1, QROWS], f32r, name="b2r_t")
            nc.gpsimd.dma_start(out=b2r_t[:], in_=b2r_d)

        x_starts = {}
        acc = 0
        for eng, n_chunks in X_SCHED:
            x_starts[acc] = (eng, n_chunks)
            acc += n_chunks
        assert acc == N_CH
        xt_base = 0
        prev = None  # h_sb tile of previous chunk
        y_ps = None
        y_sb = None
        for s in range(N_CH + 1):
            cur = None
            cur_xt = None
            if s < N_CH:
                if s in x_starts:
                    eng, n_chunks = x_starts[s]
                    xt_t = x_pool.tile(
                        [N_IN, n_chunks * CHUNK], x_dt, name="xt_t", tag="xt"
                    )
                    q0 = s * CHUNK
                    getattr(nc, eng).dma_start(
                        out=xt_t[:], in_=xt_d[:, q0 : q0 + n_chunks * CHUNK]
                    )
                    xt_base = s
                h_ps = hps_pool.tile([N_MID, CHUNK], f32, name="h_ps", tag="hps")
                base = (s - xt_base) * CHUNK
                for q in range(CHUNK // QROWS):
                    nc.tensor.matmul(
                        h_ps[:, q * QROWS : (q + 1) * QROWS],
                        w1_t[:],
                        xt_t[:, base + q * QROWS : base + (q + 1) * QROWS],
                        start=True,
                        stop=True,
                    )
                cur = hsb_pool.tile([N_MID, CHUNK], bf16, name="h_sb", tag="hsb")
                if s >= N_CH - 2:
                    nc.scalar.activation(
                        cur[:, :QROWS],
                        h_ps[:, :QROWS],
                        mybir.ActivationFunctionType.Relu,
                        bias=b1_t[:],
                    )
                    nc.vector.tensor_scalar(
                        cur[:, QROWS:], h_ps[:, QROWS:], b1_t[:], 0.0, add, mx
                    )
                elif relu_eng[s]:
                    nc.scalar.activation(
                        cur[:],
                        h_ps[:],
                        mybir.ActivationFunctionType.Relu,
                        bias=b1_t[:],
                    )
                else:
                    nc.vector.tensor_scalar(cur[:], h_ps[:], b1_t[:], 0.0, add, mx)
            if s >= 1:
                t = s - 1
                g = t // 2
                if t % 2 == 0:
                    y_ps = yps_pool.tile([N_MID, QROWS], f32, name="y_ps", tag="yps")
                    if acty[g]:
                        # pre-load the bank with the tiled b2 pattern
                        # (exact f32 payload via the f32r instruction dtype)
                        nc.tensor.matmul(
                            y_ps[:],
                            onesr_t[:],
                            b2r_t[:],
                            start=True,
                            stop=True,
                            skip_group_check=True,
                        )
                off = (t % 2) * (CHUNK // N_MID) * N_OUT  # 0 or 256
                for j in range(CHUNK // N_MID):  # 8 row-blocks of 128
                    blk = prev[:, j * N_MID : (j + 1) * N_MID]
                    nc.tensor.matmul(
                        y_ps[:, off + j * N_OUT : off + (j + 1) * N_OUT],
                        blk,
                        w2_t[:],
                        start=not acty[g],
                        stop=True,
                        skip_group_check=True,
                    )
                if t % 2 == 1:
                    half = g % 2
                    if half == 0:
                        y_sb = ysb_pool.tile(
                            [N_MID, 2 * QROWS], y_dt, name="y_sb", tag="ysb"
                        )
                    y_dst = y_sb[:, half * QROWS : (half + 1) * QROWS]
                    if acty[g]:
                        nc.scalar.activation(
                            y_dst, y_ps[:], mybir.ActivationFunctionType.Copy
                        )
                    else:
                        nc.vector.tensor_tensor(y_dst, y_ps[:], b2t_t[:], add)
                    if g == N_YG - 1:
                        # final group: two half-DMAs on both queues in parallel
                        hq = QROWS // 2
                        c0 = half * QROWS
                        nc.sync.dma_start(
                            out=y_d[g // 2, :, c0 : c0 + hq],
                            in_=y_sb[:, c0 : c0 + hq],
                        )
                        nc.gpsimd.dma_start(
                            out=y_d[g // 2, :, c0 + hq : c0 + QROWS],
                            in_=y_sb[:, c0 + hq : c0 + QROWS],
                        )
                    elif g == N_YG - 2:
                        # drain: ship each remaining group on its own queue
                        y_eng = nc.sync if g % 2 == 0 else nc.gpsimd
                        y_eng.dma_start(
                            out=y_d[g // 2, :, half * QROWS : (half + 1) * QROWS],
                            in_=y_dst,
                        )
                    elif half == 1:
                        y_eng = nc.sync if (g // 2) % 2 == 0 else nc.gpsimd
                        y_eng.dma_start(out=y_d[g // 2], in_=y_sb[:])
            prev = cur

    nc.compile()
    return nc


def _get_nc(mode: str = MODE, act_relu: int = ACT_RELU, act_y: int = ACT_Y):
    key = (mode, act_relu, act_y)
    if key not in _CACHE:
        _CACHE[key] = _build_nc(mode, act_relu, act_y)
    return _CACHE[key]


def _prep_in_maps(x, W1, b1, W2, b2, mode: str = MODE):
    import ml_dtypes

    x_np = ml_dtypes.bfloat16 if mode == "fast" else np.float32
    x = np.ascontiguousarray(x, dtype=np.float32)
    # [8, 64, B_C] feature-major shards
    xt = np.ascontiguousarray(
        x.reshape(N_CORES, B_C, N_IN).transpose(0, 2, 1).astype(x_np)
    )
    w1 = np.ascontiguousarray(W1, dtype=np.float32).astype(x_np)
    w2 = np.ascontiguousarray(W2, dtype=np.float32).astype(ml_dtypes.bfloat16)
    b1c = np.ascontiguousarray(b1, dtype=np.float32).reshape(N_MID, 1)
    b2f = np.asarray(b2, dtype=np.float32)
    # b2 tiled along the free dim: b2t[p, 32*j + o] = b2[o]
    b2t = np.ascontiguousarray(
        np.tile(b2f, (N_MID, QROWS // N_OUT)).astype(ml_dtypes.bfloat16)
    )
    b2r = np.ascontiguousarray(np.tile(b2f, QROWS // N_OUT).reshape(1, QROWS))
    onesr = np.ones((1, N_MID), dtype=np.float32)
    return [
        {
            "xt": xt[i],
            "w1": w1,
            "b1": b1c,
            "w2": w2,
            "b2t": b2t,
            "onesr": onesr,
            "b2r": b2r,
        }
        for i in range(N_CORES)
    ]


def _unshard(results):
    outs = []
    for i in range(N_CORES):
        yd = np.asarray(results[i]["y"], dtype=np.float32)  # [8, 128, 1024]
        # yd[gp, p, 512*half + 256*u + 32*j + o]
        #   = y[4096*gp + 2048*half + 1024*u + 128*j + p, o]
        y = (
            yd.reshape(N_YG // 2, N_MID, 2, 2, CHUNK // N_MID, N_OUT)
            .transpose(0, 2, 3, 4, 1, 5)
            .reshape(B_C, N_OUT)
        )
        outs.append(y)
    return np.ascontiguousarray(np.concatenate(outs, axis=0))


def run(x, W1, b1, W2, b2, trace=False, mode: str = MODE):
    from concourse.bass_utils import run_bass_kernel_spmd

    nc = _get_nc(mode)
    in_maps = _prep_in_maps(x, W1, b1, W2, b2, mode)
    res = run_bass_kernel_spmd(nc, in_maps, list(range(N_CORES)), trace=trace)
    return _unshard(res.results), res


def kernel(x, W1, b1, W2, b2):
    y, _ = run(x, W1, b1, W2, b2, trace=False)
    return y


# revision 35
# speedup vs baseline: 873602.7094x; 1.0063x over previous
"""Trainium2 Bass kernel: 2-layer MLP forward  y = relu(x@W1 + b1) @ W2 + b2.

Shapes: x [262144, 64], W1 [64, 128], b1 [128], W2 [128, 32], b2 [32].
Pure data parallel over 8 NeuronCores, 32768 rows per core.

Per-core dataflow (32 chunks of 1024 rows):
  * Host pre-transposes the x shard to feature-major xt [64, 32768] bf16.
  * xt is DMA'd on a ramped schedule (small segments first for fast
    pipeline fill, 4-chunk segments for steady state), interleaved
    between the SP (HWDGE) and Pool (SWDGE) queues which issue
    concurrently.
  * mm1 (W1 stationary, xt moving): h_ps [128 mid, 1024 rows] PSUM tile.
  * relu+b1 PSUM->SBUF bf16, round-robined between ScalarE (activation
    with per-partition bias) and VectorE (tensor_scalar (h+b1) max 0)
    to balance engine load (GPSIMD has no PSUM port, it cannot help).
  * mm2 (h block stationary, W2 moving): lhsT = h_sb[:, 128j:128j+128],
    rhs = W2 [128, 32] -> y_ps[128 rows, 32]: 32 PE cycles per 128 rows
    instead of 128 (the moving operand is tiny W2, not h).
  * y: each PSUM bank holds 2 chunks (2048 rows) of outputs; DVE adds a
    pre-tiled b2 pattern (tensor_tensor) writing bf16 to SBUF; two banks
    are batched per output DMA, alternating between the SP and Pool
    (SWDGE) queues -- per-queue DMA issue cost is ~2x the transfer time
    in the cost model, so both free queues are used. The final groups
    ship as smaller parallel DMAs to shorten the drain.
  * Chunks are software-pipelined: mm2 for chunk s-1 issues after mm1
    for chunk s, so the PE never waits on the relu engines.
"""

import os
import sys

import numpy as np

if "/opt/trn_rl_repo" not in sys.path:
    sys.path.insert(0, "/opt/trn_rl_repo")

N_CORES = 8
B = 262144
B_C = B // N_CORES  # 32768
N_IN, N_MID, N_OUT = 64, 128, 32
CHUNK = 1024  # rows per chunk (one 2-bank h PSUM tile)
QROWS = 512  # rows per mm1 matmul / y PSUM bank free dim
N_CH = B_C // CHUNK  # 32 chunks
N_YG = B_C // (2 * CHUNK)  # 16 y groups (one PSUM bank per 2 chunks)
# xt DMA schedule: (queue, chunks) in chunk order; SP and Pool run concurrently
X_SCHED = [
    ("sync", 1), ("sync", 1), ("gpsimd", 2), ("sync", 2), ("gpsimd", 2),
    ("sync", 4), ("gpsimd", 4), ("sync", 4), ("gpsimd", 4), ("sync", 4),
    ("gpsimd", 4),
]

# precision mode: "fast" = bf16 x / bf16 y, "precise" = f32r x / f32 y
MODE = os.environ.get("BASS_MLP_MODE", "fast")
# number of relu tiles (out of N_CH) handled by ACT; rest go to DVE
ACT_RELU = int(os.environ.get("BASS_MLP_ACT_RELU", "22"))
# optional explicit overrides (lists) for tuning; None -> derived defaults
RELU_PATTERN = None  # list[bool] of len N_CH: True -> ACT
Y_QUEUE = None  # list[str] of len N_YG//2: "sync" | "gpsimd" per pair-DMA

_CACHE: dict = {}


def _spread(n_slots: int, n_pick: int) -> list:
    """Evenly spread n_pick True slots over n_slots (Bresenham)."""
    out, err = [], 0
    for _ in range(n_slots):
        err += n_pick
        if err >= n_slots:
            err -= n_slots
            out.append(True)
        else:
            out.append(False)
    return out


def _build_nc(mode: str, act_relu: int):
    from contextlib import ExitStack

    import concourse.bass as bass  # noqa: F401
    import concourse.tile as tile
    from concourse import bacc, mybir

    f32 = mybir.dt.float32
    bf16 = mybir.dt.bfloat16
    x_dt = bf16 if mode == "fast" else mybir.dt.float32r
    y_dt = bf16 if mode == "fast" else f32
    add = mybir.AluOpType.add
    mx = mybir.AluOpType.max

    relu_eng = (
        list(RELU_PATTERN)
        if RELU_PATTERN is not None
        else _spread(N_CH, act_relu)
    )  # True -> ACT

    nc = bacc.Bacc(
        "TRN2", target_bir_lowering=False, debug=False, num_devices=N_CORES
    )
    xt_d = nc.dram_tensor("xt", [N_IN, B_C], x_dt, kind="ExternalInput").ap()
    w1_d = nc.dram_tensor("w1", [N_IN, N_MID], x_dt, kind="ExternalInput").ap()
    b1_d = nc.dram_tensor("b1", [N_MID, 1], f32, kind="ExternalInput").ap()
    w2_d = nc.dram_tensor("w2", [N_MID, N_OUT], bf16, kind="ExternalInput").ap()
    b2t_d = nc.dram_tensor("b2t", [N_MID, QROWS], bf16, kind="ExternalInput").ap()
    # pairs of y groups batched per DMA
    y_d = nc.dram_tensor(
        "y", [N_YG // 2, N_MID, 2 * QROWS], y_dt, kind="ExternalOutput"
    ).ap()

    with tile.TileContext(nc) as tc, ExitStack() as ctx:
        consts = ctx.enter_context(tc.tile_pool(name="consts", bufs=1))
        x_pool = ctx.enter_context(tc.tile_pool(name="xp", bufs=4))
        hsb_pool = ctx.enter_context(tc.tile_pool(name="hsb", bufs=5))
        ysb_pool = ctx.enter_context(tc.tile_pool(name="ysb", bufs=3))
        hps_pool = ctx.enter_context(tc.tile_pool(name="hps", bufs=3, space="PSUM"))
        yps_pool = ctx.enter_context(tc.tile_pool(name="yps", bufs=2, space="PSUM"))

        # w1/b1 lead the SP queue (needed by the first chunk); w2/b2t are
        # deferred onto the Pool queue after its first xt segment
        w1_t = consts.tile([N_IN, N_MID], x_dt, name="w1_t")
        nc.sync.dma_start(out=w1_t[:], in_=w1_d)
        w2_t = consts.tile([N_MID, N_OUT], bf16, name="w2_t")
        nc.gpsimd.dma_start(out=w2_t[:], in_=w2_d)
        b1_t = consts.tile([N_MID, 1], f32, name="b1_t")
        nc.sync.dma_start(out=b1_t[:], in_=b1_d)
        b2t_t = consts.tile([N_MID, QROWS], bf16, name="b2t_t")
        nc.gpsimd.dma_start(out=b2t_t[:], in_=b2t_d)

        x_starts = {}
        acc = 0
        for eng, n_chunks in X_SCHED:
            x_starts[acc] = (eng, n_chunks)
            acc += n_chunks
        assert acc == N_CH
        xt_base = 0
        prev = None  # h_sb tile of previous chunk
        y_ps = None
        y_sb = None
        for s in range(N_CH + 1):
            cur = None
            if s < N_CH:
                if s in x_starts:
                    eng, n_chunks = x_starts[s]
                    xt_t = x_pool.tile(
                        [N_IN, n_chunks * CHUNK], x_dt, name="xt_t", tag="xt"
                    )
                    q0 = s * CHUNK
                    getattr(nc, eng).dma_start(
                        out=xt_t[:], in_=xt_d[:, q0 : q0 + n_chunks * CHUNK]
                    )
                    xt_base = s
                h_ps = hps_pool.tile([N_MID, CHUNK], f32, name="h_ps", tag="hps")
                base = (s - xt_base) * CHUNK
                for q in range(CHUNK // QROWS):
                    nc.tensor.matmul(
                        h_ps[:, q * QROWS : (q + 1) * QROWS],
                        w1_t[:],
                        xt_t[:, base + q * QROWS : base + (q + 1) * QROWS],
                        start=True,
                        stop=True,
                    )
                cur = hsb_pool.tile([N_MID, CHUNK], bf16, name="h_sb", tag="hsb")
                if s >= N_CH - 2:
                    nc.scalar.activation(
                        cur[:, :QROWS],
                        h_ps[:, :QROWS],
                        mybir.ActivationFunctionType.Relu,
                        bias=b1_t[:],
                    )
                    nc.vector.tensor_scalar(
                        cur[:, QROWS:], h_ps[:, QROWS:], b1_t[:], 0.0, add, mx
                    )
                elif relu_eng[s]:
                    nc.scalar.activation(
                        cur[:],
                        h_ps[:],
                        mybir.ActivationFunctionType.Relu,
                        bias=b1_t[:],
                    )
                else:
                    nc.vector.tensor_scalar(cur[:], h_ps[:], b1_t[:], 0.0, add, mx)
            if s >= 1:
                t = s - 1
                g = t // 2
                if t % 2 == 0:
                    y_ps = yps_pool.tile([N_MID, QROWS], f32, name="y_ps", tag="yps")
                off = (t % 2) * (CHUNK // N_MID) * N_OUT  # 0 or 256
                for j in range(CHUNK // N_MID):  # 8 row-blocks of 128
                    blk = prev[:, j * N_MID : (j + 1) * N_MID]
                    nc.tensor.matmul(
                        y_ps[:, off + j * N_OUT : off + (j + 1) * N_OUT],
                        blk,
                        w2_t[:],
                        start=True,
                        stop=True,
                    )
                if t % 2 == 1:
                    half = g % 2
                    if half == 0:
                        y_sb = ysb_pool.tile(
                            [N_MID, 2 * QROWS], y_dt, name="y_sb", tag="ysb"
                        )
                    y_dst = y_sb[:, half * QROWS : (half + 1) * QROWS]
                    nc.vector.tensor_tensor(y_dst, y_ps[:], b2t_t[:], add)
                    if g == N_YG - 1:
                        # final group: two half-DMAs on both queues in parallel
                        hq = QROWS // 2
                        c0 = half * QROWS
                        nc.sync.dma_start(
                            out=y_d[g // 2, :, c0 : c0 + hq],
                            in_=y_sb[:, c0 : c0 + hq],
                        )
                        nc.gpsimd.dma_start(
                            out=y_d[g // 2, :, c0 + hq : c0 + QROWS],
                            in_=y_sb[:, c0 + hq : c0 + QROWS],
                        )
                    elif g == N_YG - 2:
                        # drain: ship each remaining group on its own queue
                        y_eng = nc.sync if g % 2 == 0 else nc.gpsimd
                        y_eng.dma_start(
                            out=y_d[g // 2, :, half * QROWS : (half + 1) * QROWS],
                            in_=y_dst,
                        )
                    elif half == 1:
                        if Y_QUEUE is not None:
                            y_eng = getattr(nc, Y_QUEUE[g // 2])
                        else:
                            y_eng = nc.sync if (g // 2) % 2 == 0 else nc.gpsimd
                        y_eng.dma_start(out=y_d[g // 2], in_=y_sb[:])
            prev = cur

    nc.compile()
    return nc


def _get_nc(mode: str = MODE, act_relu: int = ACT_RELU):
    key = (mode, act_relu)
    if key not in _CACHE:
        _CACHE[key] = _build_nc(mode, act_relu)
    return _CACHE[key]


def _prep_in_maps(x, W1, b1, W2, b2, mode: str = MODE):
    import ml_dtypes

    x_np = ml_dtypes.bfloat16 if mode == "fast" else np.float32
    x = np.ascontiguousarray(x, dtype=np.float32)
    # [8, 64, B_C] feature-major shards
    xt = np.ascontiguousarray(
        x.reshape(N_CORES, B_C, N_IN).transpose(0, 2, 1).astype(x_np)
    )
    w1 = np.ascontiguousarray(W1, dtype=np.float32).astype(x_np)
    w2 = np.ascontiguousarray(W2, dtype=np.float32).astype(ml_dtypes.bfloat16)
    b1c = np.ascontiguousarray(b1, dtype=np.float32).reshape(N_MID, 1)
    b2f = np.asarray(b2, dtype=np.float32)
    # b2 tiled along the free dim: b2t[p, 32*j + o] = b2[o]
    b2t = np.ascontiguousarray(
        np.tile(b2f, (N_MID, QROWS // N_OUT)).astype(ml_dtypes.bfloat16)
    )
    return [
        {"xt": xt[i], "w1": w1, "b1": b1c, "w2": w2, "b2t": b2t}
        for i in range(N_CORES)
    ]


def _unshard(results):
    outs = []
    for i in range(N_CORES):
        yd = np.asarray(results[i]["y"], dtype=np.float32)  # [8, 128, 1024]
        # yd[gp, p, 512*half + 256*u + 32*j + o]
        #   = y[4096*gp + 2048*half + 1024*u + 128*j + p, o]
        y = (
            yd.reshape(N_YG // 2, N_MID, 2, 2, CHUNK // N_MID, N_OUT)
            .transpose(0, 2, 3, 4, 1, 5)
            .reshape(B_C, N_OUT)
        )
        outs.append(y)
    return np.ascontiguousarray(np.concatenate(outs, axis=0))


def run(x, W1, b1, W2, b2, trace=False, mode: str = MODE):
    from concourse.bass_utils import run_bass_kernel_spmd

    nc = _get_nc(mode)
    in_maps = _prep_in_maps(x, W1, b1, W2, b2, mode)
    res = run_bass_kernel_spmd(nc, in_maps, list(range(N_CORES)), trace=trace)
    return _unshard(res.results), res


def kernel(x, W1, b1, W2, b2):
    y, _ = run(x, W1, b1, W2, b2, trace=False)
    return y
